# revision 11
# baseline (speedup 1.0000x reference)
"""DePatchEmbed Trainium2 kernel.

Full op: x (32, 16384, 256) f32 -> out (32, 64, 256, 256) f32 with
  out[n, c, 2*ih+pi, 2*jw+pj] = x[n, jw*128+ih, c*4+pi*2+pj]

Sharding: pure data-parallel over the batch dim — 4 examples per core on
8 NeuronCores. Per core the op is a local permutation done in one pass:

  load : x[n] -> L[ih; jw, d]  SBUF, partition = ih (1 KiB contiguous runs)
  DVE  : S[ih; cl, pi, w=2jw+pj] <- L[ih; jw, (c,pi,pj)]  (strided copies,
         data never leaves its partition)
  store: S -> out[n, c-block]  (2 KiB contiguous runs)
"""

import json

import numpy as np

import concourse.bass as bass
import concourse.bass_utils
import concourse.bass2jax
import concourse.mybir as mybir
from concourse import tile
from concourse.bass_utils import run_bass_kernel_spmd

F32 = mybir.dt.float32

# ---------------------------------------------------------------------------
# The bundled walrus accepts at most one sync-wait per instruction
# ("Too many sync wait commands" in CoreV3GenImpl::setupSyncWait), but Tile's
# kernel-tail Drain carries one wait per outstanding DMA-sem lane. Rewrite the
# BIR before compilation: split any instruction with N>1 waits into N-1
# single-wait Drains followed by the original instruction with one wait.
_ORIG_COMPILE_BIR = concourse.bass_utils.compile_bir_kernel


def _split_multiwait_bir(bir_json: bytes) -> bytes:
    bir = json.loads(bir_json)
    changed = False
    first_block = True
    for fn in bir.get("functions", []):
        for bb in fn.get("blocks", []):
            insts = bb.get("instructions", [])
            out = []
            for inst in insts:
                # Strip the entry const-pool barrier (this kernel reads no
                # const APs, so engines need not rendezvous before starting).
                if first_block and inst.get("opcode") in ("Drain", "EventSemaphore"):
                    si0 = inst.get("sync_info") or {}
                    sems = [
                        w.get("ant_name", "")
                        for w in si0.get("on_wait", []) + si0.get("on_update", [])
                    ]
                    if sems and all(s.startswith("barrier_") for s in sems):
                        changed = True
                        continue
                si = inst.get("sync_info")
                waits = si.get("on_wait", []) if si else []
                if len(waits) > 1:
                    changed = True
                    for k, w in enumerate(waits[:-1]):
                        out.append(
                            {
                                "debug": inst.get("debug", 0),
                                "engine": inst["engine"],
                                "ins": [],
                                "outs": [],
                                "is_reset_sema": False,
                                "name": f"{inst['name']}-sw{k}",
                                "opcode": "Drain",
                                "sync_info": {"on_update": [], "on_wait": [w]},
                            }
                        )
                    si["on_wait"] = [waits[-1]]
                out.append(inst)
            bb["instructions"] = out
            first_block = False
    if not changed:
        return bir_json
    return json.dumps(bir).encode()


def _patched_compile_bir_kernel(bir_json, tmpdir, neff_name="file.neff"):
    return _ORIG_COMPILE_BIR(_split_multiwait_bir(bir_json), tmpdir, neff_name)


if getattr(concourse.bass2jax.compile_bir_kernel, "__name__", "") != (
    "_patched_compile_bir_kernel"
):
    concourse.bass2jax.compile_bir_kernel = _patched_compile_bir_kernel
    concourse.bass_utils.compile_bir_kernel = _patched_compile_bir_kernel

N_CORES = 8
N_FULL = 32     # full batch
NB = N_FULL // N_CORES  # examples per core
HG = 128        # H // P
WG = 128        # W // P
C = 64          # channels
P = 2           # patch size
DIM = C * P * P             # 256 floats per patch row
LFREE = WG * DIM            # floats per partition for one example
CB = 8                      # channels per store block
NCB = C // CB
SFREE = CB * P * 256
NJB = 8                     # load chunks per example
JB = WG // NJB


# jw extents of the load DMA instructions: small first chunks so the HWDGE
# doorbell (rung only after a whole instruction's descriptors are generated,
# ~4.6 ns/desc) fires early and the engines ramp fast.
LOAD_CHUNKS = (4, 12, 16, 16, 16, 16, 16, 16, 8, 8)
assert sum(LOAD_CHUNKS) == WG

# cb0 copy batches, aligned to load-chunk boundaries; the last batches are
# tiny so the first store dispatches almost immediately after the load phase
CB0_SPLITS = (0, 32, 64, 96, 112, 120, 128)


def _build_kernel(nc: bass.Bass, x: bass.AP, out: bass.AP):
    with tile.TileContext(nc) as tc:
        with (
            tc.tile_pool(name="lpool", bufs=1) as lpool,
            tc.tile_pool(name="spool", bufs=4) as spool,
        ):
            for n in range(NB):
                xv = x[n].rearrange("(jw ih) d -> ih jw d", ih=HG)
                ov = out[n].rearrange("c (ih pi) w -> ih c (pi w)", ih=HG)
                L = lpool.tile([128, LFREE], F32, tag="L")
                lv = L.rearrange("p (jw d) -> p jw d", d=DIM)
                j0 = 0
                for sz in LOAD_CHUNKS:
                    nc.sync.dma_start(
                        out=lv[:, j0 : j0 + sz, :], in_=xv[:, j0 : j0 + sz, :]
                    )
                    j0 += sz
                lshuf = L.rearrange(
                    "p (jw c pi pj) -> p jw c pi pj", jw=WG, c=C, pi=P, pj=P
                )

                def copy_block(cb, pi, pj, j0, j1, sv):
                    src = lshuf[:, j0:j1, cb * CB : (cb + 1) * CB, pi, pj]
                    src = src.transpose([0, 2, 1])  # [p, cl, jw-range]
                    dst = sv[:, :, pi, j0:j1, pj]   # [p, cl, jw-range]
                    nc.vector.tensor_copy(out=dst, in_=src)

                for cb in range(NCB):
                    S = spool.tile([128, SFREE], F32, tag="S")
                    sv = S.rearrange(
                        "p (cl pi jw pj) -> p cl pi jw pj", cl=CB, pi=P, jw=WG, pj=P
                    )
                    sfl = S.rearrange("p (cl piw) -> p cl piw", piw=P * 256)
                    if cb == 0:
                        # jw batches aligned to load chunks: earlier batches
                        # overlap the tail load DMAs; the final tiny batch is
                        # all that separates load-end from the first store
                        for q in range(len(CB0_SPLITS) - 1):
                            for pi in range(P):
                                for pj in range(P):
                                    copy_block(
                                        cb, pi, pj,
                                        CB0_SPLITS[q], CB0_SPLITS[q + 1], sv,
                                    )
                    else:
                        for pi in range(P):
                            for pj in range(P):
                                copy_block(cb, pi, pj, 0, WG, sv)
                    # Single-channel store instructions: each generates fast
                    # (128 descs per doorbell) and walks HBM sequentially,
                    # which measures ~10-20% faster per descriptor than
                    # c-strided multi-channel stores. Alternate between the
                    # two HWDGE rings so neither sequencer's per-instruction
                    # dispatch cost (~0.65us) gates the store phase.
                    for c1 in range(CB):
                        eng = nc.scalar if c1 % 2 == 0 else nc.sync
                        eng.dma_start(
                            out=ov[:, cb * CB + c1 : cb * CB + c1 + 1, :],
                            in_=sfl[:, c1 : c1 + 1, :],
                        )


_NC_CACHE = None


def _get_program() -> bass.Bass:
    global _NC_CACHE
    if _NC_CACHE is None:
        nc = bass.Bass("TRN2", num_devices=N_CORES)
        x = nc.dram_tensor("x", [NB, WG * HG, DIM], F32, kind="ExternalInput")
        out = nc.dram_tensor(
            "out", [NB, C, HG * P, WG * P], F32, kind="ExternalOutput"
        )
        _build_kernel(nc, x.ap(), out.ap())
        _NC_CACHE = nc
    return _NC_CACHE


def kernel(x: np.ndarray, H=256, W=256, **_unused) -> np.ndarray:
    x = np.ascontiguousarray(x, dtype=np.float32)
    assert x.shape == (N_FULL, WG * HG, DIM), x.shape
    nc = _get_program()
    shards = np.split(x, N_CORES, axis=0)
    in_maps = [{"x": s} for s in shards]
    res = run_bass_kernel_spmd(nc, in_maps, core_ids=list(range(N_CORES)))
    outs = [np.asarray(r["out"]) for r in res.results]
    return np.concatenate(outs, axis=0)
